# revision 19
# baseline (speedup 1.0000x reference)
"""Luong attention (general) scores + softmax, distributed over 8 TRN2 cores.

Math: reference computes softmax((enc @ W.T + b) @ h).  Algebraically
  scores = enc @ (W.T @ h) + (b . h)
and the scalar (b . h) shifts every score equally, so it cancels in the
softmax.  This collapses the O(S*H^2) matmul into an O(S*H) matvec:
  weights = softmax(enc @ v),   v = W.T @ h.

Sharding (8 cores):
  - enc [32768, 1024] row-sharded: core r gets rows [4096r, 4096(r+1)).
  - W column-sharded for computing v: core r gets W[:, 128r:128(r+1)].T
    (shipped pre-transposed, [128, 1024], so partition axis = h).
  - hidden replicated.

Per core: one fused row-dot for the local v chunk; AllGather v (tiny);
32 tiles of row-dots for local scores; AllGather scores [4096]->[32768];
global softmax computed redundantly; core 0's output is returned.

Row-dot = DVE tensor_tensor(mult) into a ping-pong product buffer, then
ScalarEngine activation(Copy, accum_out) for the per-partition sum — the
two engines pipeline at ~1.1us/tile, under the ~47us enc DMA stream that
dominates the kernel.  (tensor_tensor_reduce would do it in one DVE
instruction but crashes this runtime.)  Cross-partition reductions in
the softmax use a 32x32 stream-transpose trick instead of gpsimd.
"""

import numpy as np

import concourse.bass as bass
import concourse.mybir as mybir
from concourse.bass_utils import run_bass_kernel_spmd

H = 1024
S = 32768
NCORES = 8
S_LOC = S // NCORES  # 4096 rows of enc per core
HC = H // NCORES  # 128 columns of W per core
NT = S_LOC // 128  # 32 score tiles per core
F32 = mybir.dt.float32
RG = [list(range(NCORES))]

AF = mybir.ActivationFunctionType
ALU = mybir.AluOpType


def build_bass():
    nc = bass.Bass()

    hid_ext = nc.declare_dram_parameter("hidden", [H], F32, isOutput=False)
    enc_ext = nc.declare_dram_parameter("enc", [S_LOC, H], F32, isOutput=False)
    wt_ext = nc.declare_dram_parameter("wt", [HC, H], F32, isOutput=False)
    out_ext = nc.declare_dram_parameter("out", [S], F32, isOutput=True)

    # Collective bounce buffers (collectives cannot touch kernel I/O).
    v_in = nc.dram_tensor("v_in", [HC], F32)
    v_all = nc.dram_tensor("v_all", [H], F32, addr_space="Shared")
    sc_in = nc.dram_tensor("sc_in", [S_LOC], F32)
    sc_all = nc.dram_tensor("sc_all", [S], F32, addr_space="Shared")

    from contextlib import ExitStack

    with ExitStack() as stack:
        sb = lambda name, shape: stack.enter_context(nc.sbuf_tensor(name, shape, F32))
        enc_sb = sb("enc_sb", [128, NT * H])
        hid_b = sb("hid_b", [128, H])
        wt_sb = sb("wt_sb", [128, H])
        v_b = sb("v_b", [128, H])
        prod_v = sb("prod_v", [128, H])  # product buffer for the v row-dot
        av_scr = sb("av_scr", [128, H])  # act copy scratch for the v row-dot
        prod = [sb("prodA", [128, H]), sb("prodB", [128, H])]  # tile ping-pong
        ascr = [sb("ascrA", [128, H]), sb("ascrB", [128, H])]  # act scratch
        vloc = sb("vloc", [128, 1])
        sc_sb = sb("sc_sb", [128, NT])
        scT = sb("scT", [32, 128])
        g32 = sb("g32", [32, H])  # gathered scores, s-ordered
        e32 = sb("e32", [32, H])  # exp(score - gmax)
        w32 = sb("w32", [32, H])  # normalized weights
        m_ = sb("m_", [32, 1])
        m32 = sb("m32", [32, 32])
        mT = sb("mT", [32, 32])
        gmax = sb("gmax", [32, 1])
        ngmax = sb("ngmax", [32, 1])
        sm = sb("sm", [32, 1])
        sm32 = sb("sm32", [32, 32])
        smT = sb("smT", [32, 32])
        gsum = sb("gsum", [32, 1])
        rgs = sb("rgs", [32, 1])

        NES = 8  # rotating sems for the enc stream (8 DMAs in flight)
        qE = [stack.enter_context(nc.semaphore(f"qE{i}")) for i in range(NES)]
        qS = stack.enter_context(nc.semaphore("qS"))  # sync-queue tail DMAs
        qA = stack.enter_context(nc.semaphore("qA"))  # scalar-queue DMAs
        vg = stack.enter_context(nc.semaphore("vg"))  # vector progress
        ac = stack.enter_context(nc.semaphore("ac"))  # act progress
        cc = stack.enter_context(nc.semaphore("cc"))  # gpsimd collectives
        block = stack.enter_context(nc.Block())

        # vector progress milestones (each vector instr incs vg by 1)
        VG_V = 1  # v product written
        VG_TILES = VG_V + NT  # all tile products written (tile t -> VG_V+1+t)
        VG_SCT = VG_TILES + 4  # scores transposed into scT
        # softmax chain: max-reduce, bcast, transpose, reduce, negate
        VG_NGM = VG_SCT + 5
        # sum chain: bcast, transpose, reduce, recip, final mul
        VG_OUT = VG_NGM + 5

        # act progress milestones
        AC_V = 1  # vloc accumulated
        AC_TILES = AC_V + NT  # all score columns accumulated
        AC_EXP = AC_TILES + 1

        @block.sync
        def _(sync: bass.BassEngine):
            # Hold the enc stream until the v AllGather is done: the stream
            # saturates the 16 SDMA engines, which starves the v-chain's
            # small DMAs and stalls the collective barrier for ~70us.  A
            # quiet head phase costs ~12us and keeps all 8 cores arriving
            # at the collective together.
            sync.wait_ge(cc, 1)
            # Stream the enc shard: 32 x 512 KB.  A DMA's +16 is atomic but
            # unordered vs other in-flight DMAs, so tile t uses sem t%NES
            # with at most one DMA in flight per sem.
            for t in range(NT):
                j = t % NES
                if t >= NES:
                    sync.wait_ge(qE[j], 16 * (t // NES))
                sync.dma_start(
                    out=enc_sb[:, t * H : (t + 1) * H],
                    in_=enc_ext[t * 128 : (t + 1) * 128, :],
                ).then_inc(qE[j], 16)
            # Local scores (s-ordered) -> DRAM for the AllGather.
            sync.wait_ge(vg, VG_SCT)
            sync.dma_start(
                out=sc_in.ap().rearrange("(p f) -> p f", p=32), in_=scT[:, :]
            ).then_inc(qS, 16)
            # Gathered scores -> SBUF as [32, 1024].
            sync.wait_ge(cc, 2)
            sync.dma_start(
                out=g32[:, :], in_=sc_all.ap().rearrange("(p f) -> p f", p=32)
            ).then_inc(qS, 16)
            # Final weights -> output.
            sync.wait_ge(vg, VG_OUT)
            sync.dma_start(
                out=out_ext.ap().rearrange("(p f) -> p f", p=32), in_=w32[:, :]
            ).then_inc(qS, 16)
            sync.wait_ge(qS, 48)

        @block.scalar
        def _(scalar: bass.BassScalarEngine):
            # v-chain DMAs ride the scalar HWDGE queue so they are not
            # stuck behind the enc stream on the sync queue.
            scalar.dma_start(
                out=hid_b[:, :],
                in_=hid_ext.ap().unsqueeze(0).broadcast_to([128, H]),
            ).then_inc(qA, 16)
            scalar.dma_start(out=wt_sb[:, :], in_=wt_ext[:, :]).then_inc(qA, 16)
            # v_local[j] = sum_d wt[j,d]*h[d]: sum the DVE's product rows.
            scalar.wait_ge(vg, VG_V)
            scalar.activation(
                av_scr[:, :], prod_v[:, :], AF.Copy, bias=0.0, scale=1.0,
                accum_out=vloc[:, :],
            ).then_inc(ac, 1)
            scalar.wait_ge(ac, AC_V)  # vloc write retired before SDMA reads it
            scalar.dma_start(
                out=v_in.ap().rearrange("(p f) -> p f", p=HC), in_=vloc[:, :]
            ).then_inc(qA, 16)
            scalar.wait_ge(cc, 1)
            scalar.dma_start(
                out=v_b[:, :], in_=v_all.ap().unsqueeze(0).broadcast_to([128, H])
            ).then_inc(qA, 16)
            # Per-tile: score column t = row-sums of product tile t.
            for t in range(NT):
                scalar.wait_ge(vg, VG_V + 1 + t)
                if t >= 2:
                    scalar.wait_ge(ac, AC_V + t - 1)  # scratch t%2 reusable
                scalar.activation(
                    ascr[t % 2][:, :], prod[t % 2][:, :], AF.Copy,
                    bias=0.0, scale=1.0, accum_out=sc_sb[:, t : t + 1],
                ).then_inc(ac, 1)
            # exp(score - gmax) over the gathered scores, with row sums.
            scalar.wait_ge(vg, VG_NGM)
            scalar.activation(
                e32[:, :], g32[:, :], AF.Exp, bias=ngmax[:, :], scale=1.0,
                accum_out=sm[:, :],
            ).then_inc(ac, 1)

        @block.vector
        def _(vector: bass.BassVectorEngine):
            vector.wait_ge(qA, 32)
            vector.tensor_tensor(
                out=prod_v[:, :], in0=wt_sb[:, :], in1=hid_b[:, :], op=ALU.mult
            ).then_inc(vg, 1)
            vector.wait_ge(qA, 64)  # v_b broadcast ready
            for t in range(NT):
                vector.wait_ge(qE[t % NES], 16 * (t // NES + 1))
                if t >= 2:
                    vector.wait_ge(ac, AC_V + t - 1)  # act consumed prod[t%2]
                vector.tensor_tensor(
                    out=prod[t % 2][:, :],
                    in0=enc_sb[:, t * H : (t + 1) * H],
                    in1=v_b[:, :],
                    op=ALU.mult,
                ).then_inc(vg, 1)
            # [128, 32] -> [32, 128] so DRAM scores land in s-order.
            vector.wait_ge(ac, AC_TILES)  # all score columns written
            for bb in range(4):
                vector.transpose(
                    out=scT[0:32, bb * 32 : (bb + 1) * 32],
                    in_=sc_sb[bb * 32 : (bb + 1) * 32, 0:32],
                ).then_inc(vg, 1)
            # --- global softmax stats via 32x32 transpose trick ---
            vector.wait_ge(qS, 32)  # g32 loaded
            vector.tensor_reduce(
                out=m_[:, :], in_=g32[:, :], axis=mybir.AxisListType.X, op=ALU.max
            ).then_inc(vg, 1)
            vector.wait_ge(vg, VG_SCT + 1)
            # broadcast per-partition max across 32 cols: g32*0 + m_
            vector.tensor_scalar(
                out=m32[:, :], in0=g32[:, 0:32], scalar1=0.0, scalar2=m_[:, :],
                op0=ALU.mult, op1=ALU.add,
            ).then_inc(vg, 1)
            vector.wait_ge(vg, VG_SCT + 2)
            vector.transpose(out=mT[:, :], in_=m32[:, :]).then_inc(vg, 1)
            vector.wait_ge(vg, VG_SCT + 3)
            vector.tensor_reduce(
                out=gmax[:, :], in_=mT[:, :], axis=mybir.AxisListType.X, op=ALU.max
            ).then_inc(vg, 1)
            vector.wait_ge(vg, VG_SCT + 4)
            vector.tensor_scalar_mul(ngmax[:, :], gmax[:, :], -1.0).then_inc(vg, 1)
            # --- global sum of exps, same trick ---
            vector.wait_ge(ac, AC_EXP)
            vector.tensor_scalar(
                out=sm32[:, :], in0=g32[:, 0:32], scalar1=0.0, scalar2=sm[:, :],
                op0=ALU.mult, op1=ALU.add,
            ).then_inc(vg, 1)
            vector.wait_ge(vg, VG_NGM + 1)
            vector.transpose(out=smT[:, :], in_=sm32[:, :]).then_inc(vg, 1)
            vector.wait_ge(vg, VG_NGM + 2)
            vector.tensor_reduce(
                out=gsum[:, :], in_=smT[:, :], axis=mybir.AxisListType.X, op=ALU.add
            ).then_inc(vg, 1)
            vector.wait_ge(vg, VG_NGM + 3)
            vector.reciprocal(rgs[:, :], gsum[:, :]).then_inc(vg, 1)
            vector.wait_ge(vg, VG_NGM + 4)
            vector.tensor_scalar_mul(w32[:, :], e32[:, :], rgs[:, :]).then_inc(vg, 1)

        @block.gpsimd
        def _(gpsimd: bass.BassGpSimd):
            gpsimd.wait_ge(qA, 48)
            gpsimd.collective_compute(
                "AllGather",
                ALU.bypass,
                replica_groups=RG,
                ins=[v_in.ap().opt()],
                outs=[v_all.ap().opt()],
            ).then_inc(cc, 1)
            gpsimd.wait_ge(qS, 16)
            gpsimd.collective_compute(
                "AllGather",
                ALU.bypass,
                replica_groups=RG,
                ins=[sc_in.ap().opt()],
                outs=[sc_all.ap().opt()],
            ).then_inc(cc, 1)

    # Populate .instr bytes for extended-inst subclasses — raw Bass skips
    # the Bacc.compile() pass that normally does this, and the NEFF
    # compiler fails with "ISA wrong length" without it.
    mybir.codegen_inst_isa_subclasses(nc)
    return nc


_NC_CACHE = None


def _get_nc():
    global _NC_CACHE
    if _NC_CACHE is None:
        _NC_CACHE = build_bass()
    return _NC_CACHE


def make_in_maps(hidden, encoder_outputs, W):
    hid = np.ascontiguousarray(np.asarray(hidden, dtype=np.float32).reshape(H))
    enc = np.asarray(encoder_outputs, dtype=np.float32).reshape(S, H)
    Wf = np.asarray(W, dtype=np.float32)
    in_maps = []
    for r in range(NCORES):
        in_maps.append(
            {
                "hidden": hid,
                "enc": np.ascontiguousarray(enc[r * S_LOC : (r + 1) * S_LOC]),
                "wt": np.ascontiguousarray(Wf[:, r * HC : (r + 1) * HC].T),
            }
        )
    return in_maps


def kernel(hidden, encoder_outputs, W, b):
    # b only shifts every score by the constant (b . hidden); softmax is
    # invariant to that shift, so b never needs to reach the device.
    del b
    in_maps = make_in_maps(hidden, encoder_outputs, W)
    nc = _get_nc()
    res = run_bass_kernel_spmd(nc, in_maps, core_ids=list(range(NCORES)))
    out = np.asarray(res.results[0]["out"], dtype=np.float32)
    return out.reshape(1, 1, S)


# revision 29
# speedup vs baseline: 1.3107x; 1.3107x over previous
"""Luong attention (general) scores + softmax, distributed over 8 TRN2 cores.

Math: reference computes softmax((enc @ W.T + b) @ h).  Algebraically
  scores = enc @ (W.T @ h) + (b . h)
and the scalar (b . h) shifts every score equally, so it cancels in the
softmax.  This collapses the O(S*H^2) matmul into an O(S*H) matvec:
  weights = softmax(enc @ v),   v = W.T @ h.

Sharding (8 cores):
  - enc [32768, 1024] row-sharded: core r gets rows [4096r, 4096(r+1)).
  - W replicated (4 MB), pre-arranged on host so the partition axis is h:
    wt[j, 1024c + d] = W[d, 128c + j].  Every core computes the full v
    redundantly — a v AllGather would be cheaper in bytes, but the FIRST
    collective in a NEFF pays a ~40us cold-start on this stack, which
    dominated earlier versions.  Instead a dummy warm-up collective fires
    at t~0 (nothing waits on it) so the one real collective — the score
    AllGather — runs warm (~1us trigger-to-mesh).
  - hidden replicated.

Row-dot = DVE tensor_tensor(mult) into a ping-pong product buffer, then
ScalarEngine activation(Copy, accum_out) for the per-partition sum — the
engines pipeline at ~1.4us/tile under the enc DMA stream.
(tensor_tensor_reduce would do it in one instruction but crashes this
runtime.)  Cross-partition softmax reductions use a 32x32
stream-transpose trick (gpsimd partition_all_reduce needs a ucode
library that also crashes).

v chunk c lands as a column of vcol32 [128, 32]; stream transposes give
vT [32, 128] whose rows 0..7 are v in linear order; a DRAM round trip
broadcasts it to v_b [128, 1024] for the row-dots.
"""

import numpy as np

import concourse.bass as bass
import concourse.mybir as mybir
from concourse.bass_utils import run_bass_kernel_spmd

H = 1024
S = 32768
NCORES = 8
S_LOC = S // NCORES  # 4096 rows of enc per core
NT = S_LOC // 128  # 32 score tiles per core
NC = H // 128  # 8 v chunks
F32 = mybir.dt.float32
RG = [list(range(NCORES))]

AF = mybir.ActivationFunctionType
ALU = mybir.AluOpType


def build_bass():
    nc = bass.Bass()

    hid_ext = nc.declare_dram_parameter("hidden", [H], F32, isOutput=False)
    enc_ext = nc.declare_dram_parameter("enc", [S_LOC, H], F32, isOutput=False)
    wt_ext = nc.declare_dram_parameter("wt", [128, NC * H], F32, isOutput=False)
    out_ext = nc.declare_dram_parameter("out", [S], F32, isOutput=True)

    # Collective bounce buffers (collectives cannot touch kernel I/O).
    warm_in = nc.dram_tensor("warm_in", [16], F32)
    warm_out = nc.dram_tensor("warm_out", [16 * NCORES], F32, addr_space="Shared")
    v_dram = nc.dram_tensor("v_dram", [H], F32)
    sc_in = nc.dram_tensor("sc_in", [S_LOC], F32)
    sc_all = nc.dram_tensor("sc_all", [S], F32, addr_space="Shared")

    from contextlib import ExitStack

    with ExitStack() as stack:
        sb = lambda name, shape: stack.enter_context(nc.sbuf_tensor(name, shape, F32))
        enc_sb = sb("enc_sb", [128, NT * H])
        wt_sb = sb("wt_sb", [128, NC * H])
        hid_b = sb("hid_b", [128, H])
        v_b = sb("v_b", [128, H])
        prod = [sb("prodA", [128, H]), sb("prodB", [128, H])]  # DVE->Act ping-pong
        ascr = [sb("ascrA", [128, H]), sb("ascrB", [128, H])]  # act copy scratch
        vcol32 = sb("vcol32", [128, 32])  # v chunks as columns (cols 0..7)
        vT = sb("vT", [32, 128])  # rows 0..7 = v in linear order
        z16 = sb("z16", [1, 16])
        sc_sb = sb("sc_sb", [128, NT])
        scT = sb("scT", [32, 128])
        g32 = sb("g32", [32, H])  # gathered scores, s-ordered
        e32 = sb("e32", [32, H])  # exp(score - gmax)
        w32 = sb("w32", [32, H])  # normalized weights
        m_ = sb("m_", [32, 1])
        m32 = sb("m32", [32, 32])
        mT = sb("mT", [32, 32])
        gmax = sb("gmax", [32, 1])
        ngmax = sb("ngmax", [32, 1])
        sm = sb("sm", [32, 1])
        sm32 = sb("sm32", [32, 32])
        smT = sb("smT", [32, 32])
        gsum = sb("gsum", [32, 1])
        rgs = sb("rgs", [32, 1])

        NES = 8  # rotating sems for the enc stream (8 DMAs in flight)
        qE = [stack.enter_context(nc.semaphore(f"qE{i}")) for i in range(NES)]
        qS = stack.enter_context(nc.semaphore("qS"))  # sync-queue tail DMAs
        qA = stack.enter_context(nc.semaphore("qA"))  # scalar-queue DMAs
        qW = stack.enter_context(nc.semaphore("qW"))  # sync-queue wt half
        qH = stack.enter_context(nc.semaphore("qH"))  # hid_b broadcast DMA
        qG = stack.enter_context(nc.semaphore("qG"))  # gpsimd warm-in DMA
        qZ = stack.enter_context(nc.semaphore("qZ"))  # z16 memset done
        vg = stack.enter_context(nc.semaphore("vg"))  # vector progress
        ac = stack.enter_context(nc.semaphore("ac"))  # act progress
        cc = stack.enter_context(nc.semaphore("cc"))  # gpsimd collectives
        block = stack.enter_context(nc.Block())

        # vector progress milestones (each vector instr incs vg by 1)
        VG_MEMSET = 1
        VG_VMULT0 = 2  # v-chunk mult c retires at VG_VMULT0 + c
        VG_VT = VG_VMULT0 + NC + 3  # 4 vT transposes: ..10..13
        VG_TILE0 = VG_VT + 1  # tile mult t retires at VG_TILE0 + t
        VG_SCT = VG_TILE0 + NT + 3  # 4 scT transposes: ..46..49
        VG_NGM = VG_SCT + 5  # max-reduce, bcast, transpose, reduce, negate
        VG_OUT = VG_NGM + 5  # bcast, transpose, reduce, recip, final mul

        # act progress: v-copy c -> c+1; tile copy t -> NC+1+t; exp last
        AC_VCOPIES = NC  # 8
        AC_TILES = NC + NT  # 40
        AC_EXP = AC_TILES + 1

        @block.sync
        def _(sync: bass.BassEngine):
            # First half of wt (chunks 0..3) — needed by the v row-dots.
            sync.dma_start(out=wt_sb[:, : 4 * H], in_=wt_ext[:, : 4 * H]).then_inc(
                qW, 16
            )
            # Head of the enc stream: 4 tiles so the DVE never starves when
            # v_b lands.
            for t in range(4):
                sync.dma_start(
                    out=enc_sb[:, t * H : (t + 1) * H],
                    in_=enc_ext[t * 128 : (t + 1) * 128, :],
                ).then_inc(qE[t], 16)
            # Hold the rest until the v-chain's small DMAs are through the
            # queues — the full stream starves them by ~10x otherwise.
            sync.wait_ge(qA, 32)
            for t in range(4, NT):
                j = t % NES
                if t >= NES:
                    sync.wait_ge(qE[j], 16 * (t // NES))
                sync.dma_start(
                    out=enc_sb[:, t * H : (t + 1) * H],
                    in_=enc_ext[t * 128 : (t + 1) * 128, :],
                ).then_inc(qE[j], 16)
            # Local scores (s-ordered) -> DRAM for the AllGather.
            sync.wait_ge(vg, VG_SCT)
            sync.dma_start(
                out=sc_in.ap().rearrange("(p f) -> p f", p=32), in_=scT[:, :]
            ).then_inc(qS, 16)
            # Gathered scores -> SBUF as [32, 1024].
            sync.wait_ge(cc, 2)
            sync.dma_start(
                out=g32[:, :], in_=sc_all.ap().rearrange("(p f) -> p f", p=32)
            ).then_inc(qS, 16)
            # Final weights -> output.
            sync.wait_ge(vg, VG_OUT)
            sync.dma_start(
                out=out_ext.ap().rearrange("(p f) -> p f", p=32), in_=w32[:, :]
            ).then_inc(qS, 16)
            sync.wait_ge(qS, 48)

        @block.scalar
        def _(scalar: bass.BassScalarEngine):
            scalar.dma_start(
                out=hid_b[:, :],
                in_=hid_ext.ap().unsqueeze(0).broadcast_to([128, H]),
            ).then_inc(qH, 16)
            # Second half of wt (chunks 4..7).
            scalar.dma_start(
                out=wt_sb[:, 4 * H :], in_=wt_ext[:, 4 * H :]
            ).then_inc(qA, 16)
            # v chunk c: column c of vcol32 = row-sums of wt chunk c * hid.
            for c in range(NC):
                scalar.wait_ge(vg, VG_VMULT0 + c)
                if c >= 2:
                    scalar.wait_ge(ac, c - 1)  # ascr[c%2] reusable
                scalar.activation(
                    ascr[c % 2][:, :], prod[c % 2][:, :], AF.Copy,
                    bias=0.0, scale=1.0, accum_out=vcol32[:, c : c + 1],
                ).then_inc(ac, 1)
            # v (linear, rows 0..7 of vT) -> DRAM, then broadcast-read back
            # to all 128 partitions.
            scalar.wait_ge(vg, VG_VT)
            scalar.dma_start(
                out=v_dram.ap().rearrange("(p f) -> p f", p=NC), in_=vT[0:NC, :]
            ).then_inc(qA, 16)
            scalar.wait_ge(qA, 32)
            scalar.dma_start(
                out=v_b[:, :], in_=v_dram.ap().unsqueeze(0).broadcast_to([128, H])
            ).then_inc(qA, 16)
            # Per-tile: score column t = row-sums of product tile t.
            for t in range(NT):
                scalar.wait_ge(vg, VG_TILE0 + t)
                scalar.wait_ge(ac, NC + t - 1)  # ascr/sc pipeline backpressure
                scalar.activation(
                    ascr[t % 2][:, :], prod[t % 2][:, :], AF.Copy,
                    bias=0.0, scale=1.0, accum_out=sc_sb[:, t : t + 1],
                ).then_inc(ac, 1)
            # exp(score - gmax) over the gathered scores, with row sums.
            scalar.wait_ge(vg, VG_NGM)
            scalar.activation(
                e32[:, :], g32[:, :], AF.Exp, bias=ngmax[:, :], scale=1.0,
                accum_out=sm[:, :],
            ).then_inc(ac, 1)

        @block.vector
        def _(vector: bass.BassVectorEngine):
            # cols 8..31 of vcol32 are read (as garbage) by the transposes
            # into vT rows 8..31, which are never consumed — but they must
            # be initialized for the simulator's NaN poisoning.
            vector.memset(vcol32[:, :], 0.0).then_inc(vg, 1)
            for c in range(NC):
                vector.wait_ge(qA if c >= 4 else qW, 16)
                if c == 0:
                    vector.wait_ge(qH, 16)  # hid_b
                if c >= 2:
                    vector.wait_ge(ac, c - 1)  # act consumed prod[c%2]
                vector.tensor_tensor(
                    out=prod[c % 2][:, :],
                    in0=wt_sb[:, c * H : (c + 1) * H],
                    in1=hid_b[:, :],
                    op=ALU.mult,
                ).then_inc(vg, 1)
            vector.wait_ge(ac, AC_VCOPIES)  # all v columns written
            for bb in range(4):
                vector.transpose(
                    out=vT[0:32, bb * 32 : (bb + 1) * 32],
                    in_=vcol32[bb * 32 : (bb + 1) * 32, 0:32],
                ).then_inc(vg, 1)
            vector.wait_ge(qA, 48)  # v_b broadcast ready
            for t in range(NT):
                vector.wait_ge(qE[t % NES], 16 * (t // NES + 1))
                vector.wait_ge(ac, NC + t - 1)  # act consumed prod[t%2]
                vector.tensor_tensor(
                    out=prod[t % 2][:, :],
                    in0=enc_sb[:, t * H : (t + 1) * H],
                    in1=v_b[:, :],
                    op=ALU.mult,
                ).then_inc(vg, 1)
            # [128, 32] -> [32, 128] so DRAM scores land in s-order.
            vector.wait_ge(ac, AC_TILES)  # all score columns written
            for bb in range(4):
                vector.transpose(
                    out=scT[0:32, bb * 32 : (bb + 1) * 32],
                    in_=sc_sb[bb * 32 : (bb + 1) * 32, 0:32],
                ).then_inc(vg, 1)
            # --- global softmax stats via 32x32 transpose trick ---
            vector.wait_ge(qS, 32)  # g32 loaded
            vector.tensor_reduce(
                out=m_[:, :], in_=g32[:, :], axis=mybir.AxisListType.X, op=ALU.max
            ).then_inc(vg, 1)
            vector.wait_ge(vg, VG_SCT + 1)
            # broadcast per-partition max across 32 cols: g32*0 + m_
            vector.tensor_scalar(
                out=m32[:, :], in0=g32[:, 0:32], scalar1=0.0, scalar2=m_[:, :],
                op0=ALU.mult, op1=ALU.add,
            ).then_inc(vg, 1)
            vector.wait_ge(vg, VG_SCT + 2)
            vector.transpose(out=mT[:, :], in_=m32[:, :]).then_inc(vg, 1)
            vector.wait_ge(vg, VG_SCT + 3)
            vector.tensor_reduce(
                out=gmax[:, :], in_=mT[:, :], axis=mybir.AxisListType.X, op=ALU.max
            ).then_inc(vg, 1)
            vector.wait_ge(vg, VG_SCT + 4)
            vector.tensor_scalar_mul(ngmax[:, :], gmax[:, :], -1.0).then_inc(vg, 1)
            # --- global sum of exps, same trick ---
            vector.wait_ge(ac, AC_EXP)
            vector.tensor_scalar(
                out=sm32[:, :], in0=g32[:, 0:32], scalar1=0.0, scalar2=sm[:, :],
                op0=ALU.mult, op1=ALU.add,
            ).then_inc(vg, 1)
            vector.wait_ge(vg, VG_NGM + 1)
            vector.transpose(out=smT[:, :], in_=sm32[:, :]).then_inc(vg, 1)
            vector.wait_ge(vg, VG_NGM + 2)
            vector.tensor_reduce(
                out=gsum[:, :], in_=smT[:, :], axis=mybir.AxisListType.X, op=ALU.add
            ).then_inc(vg, 1)
            vector.wait_ge(vg, VG_NGM + 3)
            vector.reciprocal(rgs[:, :], gsum[:, :]).then_inc(vg, 1)
            vector.wait_ge(vg, VG_NGM + 4)
            vector.tensor_scalar_mul(w32[:, :], e32[:, :], rgs[:, :]).then_inc(vg, 1)

        @block.gpsimd
        def _(gpsimd: bass.BassGpSimd):
            # Dummy collective fired immediately: absorbs the ~40us
            # first-collective cold-start while the enc stream runs, so the
            # real score AllGather below starts in ~1us.
            gpsimd.memset(z16[:, :], 0.0).then_inc(qZ, 1)
            gpsimd.wait_ge(qZ, 1)
            gpsimd.dma_start(out=warm_in.ap(), in_=z16[0, :]).then_inc(qG, 16)
            gpsimd.wait_ge(qG, 16)
            gpsimd.collective_compute(
                "AllGather",
                ALU.bypass,
                replica_groups=RG,
                ins=[warm_in.ap().opt()],
                outs=[warm_out.ap().opt()],
            ).then_inc(cc, 1)
            gpsimd.wait_ge(qS, 16)
            gpsimd.collective_compute(
                "AllGather",
                ALU.bypass,
                replica_groups=RG,
                ins=[sc_in.ap().opt()],
                outs=[sc_all.ap().opt()],
            ).then_inc(cc, 1)

    # Populate .instr bytes for extended-inst subclasses — raw Bass skips
    # the Bacc.compile() pass that normally does this, and the NEFF
    # compiler fails with "ISA wrong length" without it.
    mybir.codegen_inst_isa_subclasses(nc)
    return nc


_NC_CACHE = None


def _get_nc():
    global _NC_CACHE
    if _NC_CACHE is None:
        _NC_CACHE = build_bass()
    return _NC_CACHE


_WT_CACHE = None


def make_in_maps(hidden, encoder_outputs, W):
    global _WT_CACHE
    hid = np.ascontiguousarray(np.asarray(hidden, dtype=np.float32).reshape(H))
    enc = np.asarray(encoder_outputs, dtype=np.float32).reshape(S, H)
    Wf = np.asarray(W, dtype=np.float32)
    # wt[j, 1024c + d] = W[d, 128c + j]: chunk c of v comes from columns
    # [128c, 128c+128) of W, laid out with h on the partition axis.
    wt = np.ascontiguousarray(
        Wf.T.reshape(NC, 128, H).transpose(1, 0, 2).reshape(128, NC * H)
    )
    in_maps = []
    for r in range(NCORES):
        in_maps.append(
            {
                "hidden": hid,
                "enc": np.ascontiguousarray(enc[r * S_LOC : (r + 1) * S_LOC]),
                "wt": wt,
            }
        )
    return in_maps


def kernel(hidden, encoder_outputs, W, b):
    # b only shifts every score by the constant (b . hidden); softmax is
    # invariant to that shift, so b never needs to reach the device.
    del b
    in_maps = make_in_maps(hidden, encoder_outputs, W)
    nc = _get_nc()
    res = run_bass_kernel_spmd(nc, in_maps, core_ids=list(range(NCORES)))
    out = np.asarray(res.results[0]["out"], dtype=np.float32)
    return out.reshape(1, 1, S)


# revision 31
# speedup vs baseline: 1.4869x; 1.1344x over previous
"""Luong attention (general) scores + softmax, distributed over 8 TRN2 cores.

Math: reference computes softmax((enc @ W.T + b) @ h).  Algebraically
  scores = enc @ (W.T @ h) + (b . h)
and the scalar (b . h) shifts every score equally, so it cancels in the
softmax.  This collapses the O(S*H^2) matmul into an O(S*H) matvec:
  weights = softmax(enc @ v),   v = W.T @ h.

Sharding (8 cores):
  - enc [32768, 1024] row-sharded: core r gets rows [4096r, 4096(r+1)).
  - W replicated (4 MB), pre-arranged on host so the partition axis is h:
    wt[j, 1024c + d] = W[d, 128c + j].  Every core computes the full v
    redundantly — a v AllGather would be cheaper in bytes, but the FIRST
    collective in a NEFF pays a ~40-50us cold-start on this stack (ncfw
    init), which would sit on the v critical path.  Instead a dummy
    warm-up collective fires at ~12us (nothing waits on it) so the one
    real collective — the score AllGather — runs warm.
  - hidden replicated.

Row-dot = one DVE scalar_tensor_tensor per tile:
  out = (enc*1.0)*v_b elementwise, accum_out = per-partition row sums.
(tensor_tensor_reduce would be the canonical op but crashes this
runtime; scalar_tensor_tensor with op0=mult/scalar=1 is equivalent.)
The ScalarEngine only pre-warms the Exp table and runs the final Exp.
Cross-partition softmax reductions use a 32x32 stream-transpose trick
(gpsimd partition_all_reduce needs a ucode library that also crashes).

v chunk c lands as a column of vcol32 [128, 32]; stream transposes give
vT [32, 128] whose rows 0..7 are v in linear order; a DRAM round trip
broadcasts it to v_b [128, 1024] for the row-dots.
"""

import numpy as np

import concourse.bass as bass
import concourse.mybir as mybir
from concourse.bass_utils import run_bass_kernel_spmd

H = 1024
S = 32768
NCORES = 8
S_LOC = S // NCORES  # 4096 rows of enc per core
NT = S_LOC // 128  # 32 score tiles per core
NC = H // 128  # 8 v chunks
F32 = mybir.dt.float32
RG = [list(range(NCORES))]

AF = mybir.ActivationFunctionType
ALU = mybir.AluOpType


def build_bass():
    nc = bass.Bass()

    hid_ext = nc.declare_dram_parameter("hidden", [H], F32, isOutput=False)
    enc_ext = nc.declare_dram_parameter("enc", [S_LOC, H], F32, isOutput=False)
    wt_ext = nc.declare_dram_parameter("wt", [128, NC * H], F32, isOutput=False)
    out_ext = nc.declare_dram_parameter("out", [S], F32, isOutput=True)

    # Collective bounce buffers (collectives cannot touch kernel I/O).
    warm_in = nc.dram_tensor("warm_in", [16], F32)
    warm_out = nc.dram_tensor("warm_out", [16 * NCORES], F32, addr_space="Shared")
    v_dram = nc.dram_tensor("v_dram", [H], F32)
    sc_in = nc.dram_tensor("sc_in", [S_LOC], F32)
    sc_all = nc.dram_tensor("sc_all", [S], F32, addr_space="Shared")

    from contextlib import ExitStack

    with ExitStack() as stack:
        sb = lambda name, shape: stack.enter_context(nc.sbuf_tensor(name, shape, F32))
        enc_sb = sb("enc_sb", [128, NT * H])
        wt_sb = sb("wt_sb", [128, NC * H])
        hid_b = sb("hid_b", [128, H])
        v_b = sb("v_b", [128, H])
        scr = [sb("scrA", [128, H]), sb("scrB", [128, H])]  # STT discard output
        vcol32 = sb("vcol32", [128, 32])  # v chunks as columns (cols 0..7)
        vT = sb("vT", [32, 128])  # rows 0..7 = v in linear order
        z16 = sb("z16", [1, 16])
        warm1 = sb("warm1", [1, 1])
        sc_sb = sb("sc_sb", [128, NT])
        scT = sb("scT", [32, 128])
        g32 = sb("g32", [32, H])  # gathered scores, s-ordered
        e32 = sb("e32", [32, H])  # exp(score - gmax)
        w32 = sb("w32", [32, H])  # normalized weights
        m_ = sb("m_", [32, 1])
        m32 = sb("m32", [32, 32])
        mT = sb("mT", [32, 32])
        gmax = sb("gmax", [32, 1])
        ngmax = sb("ngmax", [32, 1])
        sm = sb("sm", [32, 1])
        sm32 = sb("sm32", [32, 32])
        smT = sb("smT", [32, 32])
        gsum = sb("gsum", [32, 1])
        rgs = sb("rgs", [32, 1])

        NES = 8  # rotating sems for the enc stream (8 DMAs in flight)
        qE = [stack.enter_context(nc.semaphore(f"qE{i}")) for i in range(NES)]
        qS = stack.enter_context(nc.semaphore("qS"))  # sync-queue tail DMAs
        qG = stack.enter_context(nc.semaphore("qG"))  # warm_in init DMA
        qW = stack.enter_context(nc.semaphore("qW"))  # wt chunk 0-1 (sync)
        qW2 = stack.enter_context(nc.semaphore("qW2"))  # wt chunk 2-3 (sync)
        qA = stack.enter_context(nc.semaphore("qA"))  # wt chunk 4-5 (scalar)
        qA2 = stack.enter_context(nc.semaphore("qA2"))  # wt chunk 6-7 (scalar)
        qH = stack.enter_context(nc.semaphore("qH"))  # hid_b broadcast DMA
        qV = stack.enter_context(nc.semaphore("qV"))  # v_dram writeback
        qV2 = stack.enter_context(nc.semaphore("qV2"))  # v_b broadcast
        vg = stack.enter_context(nc.semaphore("vg"))  # vector progress
        ac = stack.enter_context(nc.semaphore("ac"))  # act progress
        cc = stack.enter_context(nc.semaphore("cc"))  # gpsimd collectives
        block = stack.enter_context(nc.Block())

        # vector progress milestones (each vector instr incs vg by 1)
        VG_Z = 1  # z16 memset
        VG_VCOL = 2  # vcol32 memset
        VG_V0 = 3  # v-chunk STT c retires at VG_V0 + c
        VG_VT = VG_V0 + NC + 3  # 4 vT transposes: 11..14
        VG_TILE0 = VG_VT + 1  # tile STT t retires at VG_TILE0 + t
        VG_SCT = VG_TILE0 + NT + 3  # 4 scT transposes: 47..50
        VG_NGM = VG_SCT + 5  # max-reduce, bcast, transpose, reduce, negate
        VG_OUT = VG_NGM + 5  # bcast, transpose, reduce, recip, final mul

        # enc head tiles issued before the v_dram/v_b round trip
        HEAD = 10

        def enc_dma(sync, t):
            j = t % NES
            if t >= NES:
                sync.wait_ge(qE[j], 16 * (t // NES))
            sync.dma_start(
                out=enc_sb[:, t * H : (t + 1) * H],
                in_=enc_ext[t * 128 : (t + 1) * 128, :],
            ).then_inc(qE[j], 16)

        @block.sync
        def _(sync: bass.BassEngine):
            # Seed the warm-up collective's input (content irrelevant).
            sync.wait_ge(vg, VG_Z)
            sync.dma_start(out=warm_in.ap(), in_=z16[0, :]).then_inc(qG, 16)
            # wt chunks 0-1 and 2-3 (the other half rides the scalar queue).
            sync.dma_start(out=wt_sb[:, : 2 * H], in_=wt_ext[:, : 2 * H]).then_inc(
                qW, 16
            )
            sync.dma_start(
                out=wt_sb[:, 2 * H : 4 * H], in_=wt_ext[:, 2 * H : 4 * H]
            ).then_inc(qW2, 16)
            # Head of the enc stream, then the v round trip on this same
            # queue (it is idle by then), then the rest of the stream.
            for t in range(HEAD):
                enc_dma(sync, t)
            sync.wait_ge(vg, VG_VT)
            sync.dma_start(
                out=v_dram.ap().rearrange("(p f) -> p f", p=NC), in_=vT[0:NC, :]
            ).then_inc(qV, 16)
            sync.wait_ge(qV, 16)
            sync.dma_start(
                out=v_b[:, :], in_=v_dram.ap().unsqueeze(0).broadcast_to([128, H])
            ).then_inc(qV2, 16)
            for t in range(HEAD, NT):
                enc_dma(sync, t)
            # Local scores (s-ordered) -> DRAM for the AllGather.
            sync.wait_ge(vg, VG_SCT)
            sync.dma_start(
                out=sc_in.ap().rearrange("(p f) -> p f", p=32), in_=scT[:, :]
            ).then_inc(qS, 16)
            # Gathered scores -> SBUF as [32, 1024].
            sync.wait_ge(cc, 2)
            sync.dma_start(
                out=g32[:, :], in_=sc_all.ap().rearrange("(p f) -> p f", p=32)
            ).then_inc(qS, 16)
            # Final weights -> output.
            sync.wait_ge(vg, VG_OUT)
            sync.dma_start(
                out=out_ext.ap().rearrange("(p f) -> p f", p=32), in_=w32[:, :]
            ).then_inc(qS, 16)
            sync.wait_ge(qS, 48)

        @block.scalar
        def _(scalar: bass.BassScalarEngine):
            scalar.dma_start(
                out=hid_b[:, :],
                in_=hid_ext.ap().unsqueeze(0).broadcast_to([128, H]),
            ).then_inc(qH, 16)
            scalar.dma_start(
                out=wt_sb[:, 4 * H : 6 * H], in_=wt_ext[:, 4 * H : 6 * H]
            ).then_inc(qA, 16)
            scalar.dma_start(
                out=wt_sb[:, 6 * H :], in_=wt_ext[:, 6 * H :]
            ).then_inc(qA2, 16)
            # Pre-load the Exp activation table off the critical path.
            scalar.wait_ge(qH, 16)
            scalar.activation(warm1[:, :], hid_b[0:1, 0:1], AF.Exp)
            # exp(score - gmax) over the gathered scores, with row sums.
            scalar.wait_ge(vg, VG_NGM)
            scalar.activation(
                e32[:, :], g32[:, :], AF.Exp, bias=ngmax[:, :], scale=1.0,
                accum_out=sm[:, :],
            ).then_inc(ac, 1)

        @block.vector
        def _(vector: bass.BassVectorEngine):
            vector.memset(z16[:, :], 0.0).then_inc(vg, 1)
            # cols 8..31 of vcol32 feed (unconsumed) vT rows 8..31, but the
            # simulator NaN-poisons uninitialized SBUF.
            vector.memset(vcol32[:, :], 0.0).then_inc(vg, 1)
            WTQ = [qW, qW, qW2, qW2, qA, qA, qA2, qA2]
            for c in range(NC):
                vector.wait_ge(qH, 16)
                vector.wait_ge(WTQ[c], 16)
                if c >= 2:
                    vector.wait_ge(vg, VG_V0 + c - 2)  # scr[c%2] retired
                else:
                    vector.wait_ge(vg, VG_VCOL)  # vcol32 memset retired
                vector.scalar_tensor_tensor(
                    out=scr[c % 2][:, :],
                    in0=wt_sb[:, c * H : (c + 1) * H],
                    scalar=1.0,
                    in1=hid_b[:, :],
                    op0=ALU.mult,
                    op1=ALU.mult,
                    accum_out=vcol32[:, c : c + 1],
                ).then_inc(vg, 1)
            vector.wait_ge(vg, VG_V0 + NC - 1)  # all v columns retired
            for bb in range(4):
                vector.transpose(
                    out=vT[0:32, bb * 32 : (bb + 1) * 32],
                    in_=vcol32[bb * 32 : (bb + 1) * 32, 0:32],
                ).then_inc(vg, 1)
            for t in range(NT):
                vector.wait_ge(qV2, 16)  # v_b broadcast ready
                vector.wait_ge(qE[t % NES], 16 * (t // NES + 1))
                if t >= 2:
                    vector.wait_ge(vg, VG_TILE0 + t - 2)  # scr[t%2] retired
                vector.scalar_tensor_tensor(
                    out=scr[t % 2][:, :],
                    in0=enc_sb[:, t * H : (t + 1) * H],
                    scalar=1.0,
                    in1=v_b[:, :],
                    op0=ALU.mult,
                    op1=ALU.mult,
                    accum_out=sc_sb[:, t : t + 1],
                ).then_inc(vg, 1)
            # [128, 32] -> [32, 128] so DRAM scores land in s-order.
            vector.wait_ge(vg, VG_TILE0 + NT - 1)  # all score columns retired
            for bb in range(4):
                vector.transpose(
                    out=scT[0:32, bb * 32 : (bb + 1) * 32],
                    in_=sc_sb[bb * 32 : (bb + 1) * 32, 0:32],
                ).then_inc(vg, 1)
            # --- global softmax stats via 32x32 transpose trick ---
            vector.wait_ge(qS, 32)  # g32 loaded
            vector.tensor_reduce(
                out=m_[:, :], in_=g32[:, :], axis=mybir.AxisListType.X, op=ALU.max
            ).then_inc(vg, 1)
            vector.wait_ge(vg, VG_SCT + 1)
            # broadcast per-partition max across 32 cols: g32*0 + m_
            vector.tensor_scalar(
                out=m32[:, :], in0=g32[:, 0:32], scalar1=0.0, scalar2=m_[:, :],
                op0=ALU.mult, op1=ALU.add,
            ).then_inc(vg, 1)
            vector.wait_ge(vg, VG_SCT + 2)
            vector.transpose(out=mT[:, :], in_=m32[:, :]).then_inc(vg, 1)
            vector.wait_ge(vg, VG_SCT + 3)
            vector.tensor_reduce(
                out=gmax[:, :], in_=mT[:, :], axis=mybir.AxisListType.X, op=ALU.max
            ).then_inc(vg, 1)
            vector.wait_ge(vg, VG_SCT + 4)
            vector.tensor_scalar_mul(ngmax[:, :], gmax[:, :], -1.0).then_inc(vg, 1)
            # --- global sum of exps, same trick ---
            vector.wait_ge(ac, 1)
            vector.tensor_scalar(
                out=sm32[:, :], in0=g32[:, 0:32], scalar1=0.0, scalar2=sm[:, :],
                op0=ALU.mult, op1=ALU.add,
            ).then_inc(vg, 1)
            vector.wait_ge(vg, VG_NGM + 1)
            vector.transpose(out=smT[:, :], in_=sm32[:, :]).then_inc(vg, 1)
            vector.wait_ge(vg, VG_NGM + 2)
            vector.tensor_reduce(
                out=gsum[:, :], in_=smT[:, :], axis=mybir.AxisListType.X, op=ALU.add
            ).then_inc(vg, 1)
            vector.wait_ge(vg, VG_NGM + 3)
            vector.reciprocal(rgs[:, :], gsum[:, :]).then_inc(vg, 1)
            vector.wait_ge(vg, VG_NGM + 4)
            vector.tensor_scalar_mul(w32[:, :], e32[:, :], rgs[:, :]).then_inc(vg, 1)

        @block.gpsimd
        def _(gpsimd: bass.BassGpSimd):
            # Dummy collective fired ASAP: absorbs the ~40-50us
            # first-collective cold-start while the stream and row-dots
            # run, so the real score AllGather below starts warm.
            gpsimd.wait_ge(qG, 16)
            gpsimd.collective_compute(
                "AllGather",
                ALU.bypass,
                replica_groups=RG,
                ins=[warm_in.ap().opt()],
                outs=[warm_out.ap().opt()],
            ).then_inc(cc, 1)
            gpsimd.wait_ge(qS, 16)
            gpsimd.collective_compute(
                "AllGather",
                ALU.bypass,
                replica_groups=RG,
                ins=[sc_in.ap().opt()],
                outs=[sc_all.ap().opt()],
            ).then_inc(cc, 1)

    # Populate .instr bytes for extended-inst subclasses — raw Bass skips
    # the Bacc.compile() pass that normally does this, and the NEFF
    # compiler fails with "ISA wrong length" without it.
    mybir.codegen_inst_isa_subclasses(nc)
    return nc


_NC_CACHE = None


def _get_nc():
    global _NC_CACHE
    if _NC_CACHE is None:
        _NC_CACHE = build_bass()
    return _NC_CACHE


def make_in_maps(hidden, encoder_outputs, W):
    hid = np.ascontiguousarray(np.asarray(hidden, dtype=np.float32).reshape(H))
    enc = np.asarray(encoder_outputs, dtype=np.float32).reshape(S, H)
    Wf = np.asarray(W, dtype=np.float32)
    # wt[j, 1024c + d] = W[d, 128c + j]: chunk c of v comes from columns
    # [128c, 128c+128) of W, laid out with h on the partition axis.
    wt = np.ascontiguousarray(
        Wf.T.reshape(NC, 128, H).transpose(1, 0, 2).reshape(128, NC * H)
    )
    in_maps = []
    for r in range(NCORES):
        in_maps.append(
            {
                "hidden": hid,
                "enc": np.ascontiguousarray(enc[r * S_LOC : (r + 1) * S_LOC]),
                "wt": wt,
            }
        )
    return in_maps


def kernel(hidden, encoder_outputs, W, b):
    # b only shifts every score by the constant (b . hidden); softmax is
    # invariant to that shift, so b never needs to reach the device.
    del b
    in_maps = make_in_maps(hidden, encoder_outputs, W)
    nc = _get_nc()
    res = run_bass_kernel_spmd(nc, in_maps, core_ids=list(range(NCORES)))
    out = np.asarray(res.results[0]["out"], dtype=np.float32)
    return out.reshape(1, 1, S)
